# revision 30
# baseline (speedup 1.0000x reference)
"""Trainium2 Bass kernel for the LoE tiled-MLP (NeRF-style coordinate net).

Sharding: data-parallel over the pixel axis. N=262144 rows are split
contiguously across 8 cores (32768 rows each). Because the per-layer
expert tiles are contiguous row blocks, each core only ever needs a
contiguous slice of every weight tensor -> zero cross-core traffic.

On-device layout: activations are feature-major [d, n]; every layer is
psum[o, n] += w[d_blk, o_blk].T @ x[d_blk, n] with w as the stationary
operand.  All layer matmuls are bf16 (tolerance 2e-2 >> bf16's ~6e-3).

The kernel keeps the PE busy ~100% of the time: TRN2's HAM clock gate
halves the PE clock (2.4 -> 1.2 GHz) whenever the PE has an idle 3.4us
window, so every PE bubble costs double.  Structure:
  * chunks run in groups of 8, layer-batched: the in-order PE queue
    holds ~6us of independent matmuls behind any wait on the LeakyReLU
    chain, and the 4-slot PSUM ring gives ~3.5us of slack before a
    slot-reuse wait can bite (worst lrelu latency ~2.8us)
  * pos-enc for group g+1 (angle matmul -> magic-round/frac on DVE ->
    sin on ACT) is pipelined into group g two chunks at a time, packed
    at partition offsets 0/64 so rnd/frac/sin are one op per 2 chunks,
    with the sin ops spread across the group so they never
    head-of-line-block the ACT queue
  * coords are pre-split on the host into three bf16 components
    (c = hi + mid + lo, exact to fp32 precision), so the angle matmul
    is one K=7 bf16 matmul instead of a 4x-slower fp32 one
  * the final [3,512] outputs land 3-to-a-psum-bank at partition
    offsets 0/32/64 (matmul tile_position), and the `last` batch is
    emitted one group late so its lrelu inputs are long since ready
  * LeakyReLU: 32 of 40 tiles/group as single ACT Prelu ops (sin and
    parametric_relu share the trig_and_small table set), 8 on DVE
    (2-op), spread 1-2 per layer so neither engine's in-order queue
    falls behind the PE's layer cadence
"""

import os
import sys

import numpy as np

sys.path.insert(0, "/opt/trn_rl_repo")

import ml_dtypes

import concourse.bass as bass
import concourse.bacc as bacc
import concourse.mybir as mybir
import concourse.tile as tile
from concourse.alu_op_type import AluOpType
from concourse.bass_utils import run_bass_kernel_spmd

F32 = mybir.dt.float32
BF16 = mybir.dt.bfloat16
ACT_SIN = mybir.ActivationFunctionType.Sin
ACT_PRELU = mybir.ActivationFunctionType.Prelu

N = 262144
NCORES = 8
ROWS = N // NCORES          # 32768 rows per core
CH = 512                    # pixels per chunk (psum free-dim, fp32 max)
G = 8                       # chunks per group
K = 13                      # frequencies
H = 256
PE_SC = 2 * 2 * K + 2       # 52 sin/cos + 2 linearized coord rows
CROWS = 7                   # coord rows: hi_x hi_y mid_x mid_y lo_x lo_y one
COORD_S = float(2.0 ** -11)  # tiny freq: sin(2*pi*s*c) ~ 2*pi*s*c
MAGIC = float(1.5 * 2 ** 23)
TWO_PI = float(2.0 * np.pi)

# which chunks of each layer's 8-chunk batch run LeakyReLU on DVE (2-op)
# instead of ACT (1-op Prelu).  8 of 40 tiles go to DVE; spread so the
# DVE queue never runs so deep that a PSUM-slot-reuse wait (ring of 4)
# stalls the PE.  Empirically tuned -- larger DVE shares and other
# spreads measured slower (356-377us vs 354us).
V_CHUNK = {0: (0, 4), 1: (2, 6), 2: (3,), 3: (3,), 4: (0, 2)}

# local (per-core) expert-tile row extents for layers 1..4
TILE_ROWS = {1: 65536, 2: 16384, 3: 4096, 4: 1024}

TRACE = False
LAST = {}


def _build(rows):
    """Build the SPMD single-core Bass program for `rows` pixels."""
    nchunks = rows // CH
    ngroups = nchunks // G
    stage = G * CH                               # coords per group
    ntile = {l: max(rows // TILE_ROWS[l], 1) for l in (1, 2, 3, 4)}
    tidx = {l: [min(j * CH // TILE_ROWS[l], ntile[l] - 1) for j in range(nchunks)]
            for l in (1, 2, 3, 4)}

    nc = bacc.Bacc()
    d_coords = nc.dram_tensor("coordsB", [CROWS, rows], BF16, kind="ExternalInput")
    d_smat = nc.dram_tensor("smat", [CROWS, PE_SC], BF16, kind="ExternalInput")
    d_w0s = nc.dram_tensor("w0s", [PE_SC, H], BF16, kind="ExternalInput")
    d_wmid = {l: nc.dram_tensor(f"w{l}", [ntile[l], H, H], BF16, kind="ExternalInput")
              for l in (1, 2, 3, 4)}
    d_wl = nc.dram_tensor("wlT", [H, 3], BF16, kind="ExternalInput")
    d_out = nc.dram_tensor("out", [3, rows], F32, kind="ExternalOutput")

    with tile.TileContext(nc) as tc:
        with (
            tc.tile_pool(name="wp", bufs=1) as wp,
            tc.tile_pool(name="io", bufs=3) as iop,
            tc.tile_pool(name="sp", bufs=10) as scp,    # sc pair tiles
            tc.tile_pool(name="fp", bufs=4) as frp,    # frac pair tiles
            tc.tile_pool(name="xp", bufs=8) as xp,     # activations
            tc.tile_pool(name="mp", bufs=5) as mp,     # rnd/rt scratch + oc out
            tc.tile_pool(name="ps", bufs=4, space="PSUM") as pp,
        ):
            # ---- resident weights (DMA once, first-use order) ----
            smat_sb = wp.tile([CROWS, PE_SC], BF16, tag="smat")
            nc.sync.dma_start(out=smat_sb[:], in_=d_smat[:])
            cr0 = iop.tile([CROWS, G * CH], BF16, tag="cr")
            nc.sync.dma_start(out=cr0[:], in_=d_coords[:, 0:G * CH])
            # w0s duplicated at partition offsets 0 and 64 so both chunks of
            # a packed sc pair tile can feed layer-0 matmuls.
            w0s_sb = wp.tile([64 + PE_SC, H], BF16, tag="w0s")
            nc.sync.dma_start(out=w0s_sb[0:PE_SC, :], in_=d_w0s[:])
            nc.sync.dma_start(out=w0s_sb[64:64 + PE_SC, :], in_=d_w0s[:])
            wl_sb = []
            for kb in range(2):
                t = wp.tile([128, 3], BF16, tag=f"wl{kb}")
                nc.sync.dma_start(out=t[:], in_=d_wl[kb * 128:(kb + 1) * 128, :])
                wl_sb.append(t)

            # HAM warm-up: junk matmuls on uninitialized SBUF keep the PE
            # busy from t=0 until the first coords land (~9us), so the
            # clock gate is already 8/8 when real work starts (cold PE
            # runs at half clock and re-warming costs a 3.4us window).
            # The junk Prelu also pulls the one-time ACT table load
            # (~1.3us, trig_and_small) off the critical path.
            junk = wp.tile([128, 128], BF16, tag="junk")
            nc.vector.memset(junk[:], 0.0)
            jps = pp.tile([128, 128], F32, tag="lps")
            for _ in range(88):
                nc.tensor.matmul(jps[:], junk[:], junk[:],
                                 start=True, stop=True)
            jact = wp.tile([128, 128], BF16, tag="jact")
            nc.scalar.activation(jact[:], jps[:], ACT_PRELU, alpha=0.2)
            cr = {}          # group -> coords tile
            sc = {}          # (group, half) -> packed sc tile [64+PE_SC, CH]
            xs = {}          # chunk -> current activation tile

            def load_coords(g):
                t = iop.tile([CROWS, stage], BF16, tag="cr")
                nc.sync.dma_start(out=t[:], in_=d_coords[:, g * stage:(g + 1) * stage])
                cr[g] = t

            def posenc_mm_rf(g, half):
                """Angle matmuls + round/frac for chunks (2*half, 2*half+1) of
                group g, packed at partition offsets 0/64 of one psum bank so
                rnd and frac are a single DVE op per pair."""
                tps = pp.tile([64 + PE_SC, CH], F32, tag="lps")
                for i in range(2):
                    o = 2 * half + i
                    rc = cr[g][:, o * CH:(o + 1) * CH]
                    nc.tensor.matmul(tps[64 * i:64 * i + PE_SC, :], smat_sb[:], rc,
                                     start=True, stop=True)
                rnd = mp.tile([64 + PE_SC, CH], F32, tag="rnd")
                nc.vector.tensor_scalar(rnd[:], tps[:], MAGIC, MAGIC,
                                        AluOpType.add, AluOpType.subtract)
                fr = frp.tile([64 + PE_SC, CH], F32, tag="fr")
                nc.vector.tensor_tensor(fr[:], tps[:], rnd[:], AluOpType.subtract)
                return fr

            def posenc_sin(g, half, fr):
                s = scp.tile([64 + PE_SC, CH], BF16, tag="sc")
                nc.scalar.activation(s[:], fr[:], ACT_SIN, scale=TWO_PI)
                sc[(g, half)] = s

            def lrelu(eng, xt, ps):
                if eng == "v":
                    rt = mp.tile([128, 2 * CH], F32, tag="rt")
                    nc.vector.tensor_scalar(rt[:], ps[:], 0.0, 0.8,
                                            AluOpType.max, AluOpType.mult)
                    nc.vector.scalar_tensor_tensor(xt[:], ps[:], 0.2, rt[:],
                                                   AluOpType.mult, AluOpType.add)
                else:
                    nc.scalar.activation(xt[:], ps[:], ACT_PRELU, alpha=0.2)

            def layer0(js):
                for i, j in enumerate(js):
                    s = sc[(j // G, i // 2)]
                    base = 64 * (i % 2)
                    ps = pp.tile([128, 2 * CH], F32, tag="lps")
                    for ob in range(2):
                        nc.tensor.matmul(ps[:, ob * CH:(ob + 1) * CH],
                                         w0s_sb[base:base + PE_SC,
                                                ob * 128:(ob + 1) * 128],
                                         s[base:base + PE_SC, :],
                                         start=True, stop=True)
                    x = xp.tile([128, 2 * CH], BF16, tag="x0")
                    lrelu("v" if i in V_CHUNK[0] else "p", x, ps)
                    xs[j] = x

            def layer(l, js):
                for i, j in enumerate(js):
                    wt = wmid_sb[l][tidx[l][j]]
                    ps = pp.tile([128, 2 * CH], F32, tag="lps")
                    for ob in range(2):
                        osl = slice(ob * CH, (ob + 1) * CH)
                        wsl = slice(ob * 128, (ob + 1) * 128)
                        for kb in range(2):
                            nc.tensor.matmul(
                                ps[:, osl], wt[kb][:, wsl],
                                xs[j][:, kb * CH:(kb + 1) * CH],
                                start=(kb == 0), stop=(kb == 1))
                    x = xp.tile([128, 2 * CH], BF16, tag=f"x{l}")
                    lrelu("v" if i in V_CHUNK[l] else "p", x, ps)
                    xs[j] = x

            def last(js, base):
                """Final [3,512] for chunks js[base:base+3], packed
                3-to-a-psum-bank at partition offsets 0/32/64 (matmul
                tile_position) -> one copy, 3 output DMAs.  Called at three
                separate emission points so neither the PE matmuls nor the
                DVE copies bunch up."""
                sub = js[base:base + 3]
                po = pp.tile([32 * (len(sub) - 1) + 3, CH], F32, tag="lps")
                for i, j in enumerate(sub):
                    for kb in range(2):
                        nc.tensor.matmul(po[32 * i:32 * i + 3, :], wl_sb[kb][:],
                                         xs[j][:, kb * CH:(kb + 1) * CH],
                                         start=(kb == 0), stop=(kb == 1))
                oc = mp.tile([32 * (len(sub) - 1) + 3, CH], F32, tag="oc")
                nc.vector.tensor_copy(oc[:], po[:])
                for i, j in enumerate(sub):
                    nc.sync.dma_start(out=d_out[:, j * CH:(j + 1) * CH],
                                      in_=oc[32 * i:32 * i + 3, :])
                    del xs[j]

            # ---- prologue: group 0's pos-enc, then bulk weights ----
            # Coords + pos-enc are emitted BEFORE the 86 expert-weight DMAs,
            # and the weight DMAs are issued from the (otherwise idle) GpSimd
            # queue: ~650ns of descriptor-issue each would otherwise hold the
            # first matmul back by ~60us of Sync-queue serialization.  The
            # group-0 coords DMA was already issued right behind smat above.
            cr[0] = cr0
            for h in range(4):
                fr = posenc_mm_rf(0, h)
                posenc_sin(0, h, fr)

            wmid_sb = {l: [[None, None] for _ in range(ntile[l])] for l in (1, 2, 3, 4)}
            order = []
            for l in (1, 2, 3, 4):
                for t in range(ntile[l]):
                    first = min(j for j in range(nchunks) if tidx[l][j] == t)
                    order.append((first, l, t))
            order.sort()
            for _, l, t in order:
                for kb in range(2):
                    w = wp.tile([128, H], BF16, tag=f"w{l}_{t}_{kb}")
                    nc.gpsimd.dma_start(
                        out=w[:], in_=d_wmid[l][t, kb * 128:(kb + 1) * 128, :])
                    wmid_sb[l][t][kb] = w

            frs = {}
            for g in range(ngroups):
                js = list(range(g * G, (g + 1) * G))
                pj = list(range((g - 1) * G, g * G))     # previous group
                nxt = g + 1 < ngroups
                if nxt:
                    load_coords(g + 1)
                    frs[0] = posenc_mm_rf(g + 1, 0)
                layer0(js)
                if g > 0:
                    last(pj, 0)
                if nxt:
                    posenc_sin(g + 1, 0, frs[0])
                    frs[1] = posenc_mm_rf(g + 1, 1)
                layer(1, js)
                if g > 0:
                    last(pj, 3)
                if nxt:
                    posenc_sin(g + 1, 1, frs[1])
                    frs[2] = posenc_mm_rf(g + 1, 2)
                layer(2, js)
                if g > 0:
                    last(pj, 6)
                if nxt:
                    posenc_sin(g + 1, 2, frs[2])
                    frs[3] = posenc_mm_rf(g + 1, 3)
                layer(3, js)
                if nxt:
                    posenc_sin(g + 1, 3, frs[3])
                    del cr[g]
                layer(4, js)
                del sc[(g, 0)], sc[(g, 1)], sc[(g, 2)], sc[(g, 3)]
            fj = list(range((ngroups - 1) * G, ngroups * G))
            for b in (0, 3, 6):
                last(fj, b)
    nc.finalize()
    return nc


def _host_prep(coords, w0, w1, w2, w3, w4, w_last, rows):
    """Split full inputs into per-core in_maps."""
    coords = np.asarray(coords, np.float32)
    bf = ml_dtypes.bfloat16
    smat = np.zeros((CROWS, PE_SC), np.float32)
    for p in range(PE_SC - 2):
        k, f, s = p >> 2, (p >> 1) & 1, p & 1
        for piece in range(3):
            smat[2 * piece + f, p] = float(2.0 ** (k - 1))
        smat[6, p] = 0.25 if s else 0.0
    for piece in range(3):
        smat[2 * piece + 0, PE_SC - 2] = COORD_S
        smat[2 * piece + 1, PE_SC - 1] = COORD_S
    w0 = np.asarray(w0, np.float32)[0]              # [54, 256]
    w0s = np.empty((PE_SC, H), np.float32)
    w0s[:PE_SC - 2] = w0[2:]
    w0s[PE_SC - 2:] = w0[0:2] / np.float32(2.0 * np.pi * COORD_S)
    wlT = np.ascontiguousarray(np.asarray(w_last, np.float32).T)  # [256, 3]
    wmid_full = {1: np.asarray(w1, np.float32), 2: np.asarray(w2, np.float32),
                 3: np.asarray(w3, np.float32), 4: np.asarray(w4, np.float32)}
    ntile = {l: max(rows // TILE_ROWS[l], 1) for l in (1, 2, 3, 4)}

    # triple bf16 split of coords: c = hi + mid + lo, exact to ~fp32
    cT = coords.T                                   # [2, N]
    hi = cT.astype(bf)
    r1 = cT - hi.astype(np.float32)
    mid = r1.astype(bf)
    lo = (r1 - mid.astype(np.float32)).astype(bf)

    in_maps = []
    for c in range(NCORES):
        sl = slice(c * rows, (c + 1) * rows)
        cb = np.empty((CROWS, rows), bf)
        cb[0:2] = hi[:, sl]
        cb[2:4] = mid[:, sl]
        cb[4:6] = lo[:, sl]
        cb[6] = np.float32(1.0)
        m = {"coordsB": cb, "smat": smat.astype(bf), "w0s": w0s.astype(bf),
             "wlT": wlT.astype(bf)}
        for l in (1, 2, 3, 4):
            w = wmid_full[l]
            t0 = (c * rows) // (N // w.shape[0])
            m[f"w{l}"] = np.ascontiguousarray(w[t0:t0 + ntile[l]]).astype(bf)
        in_maps.append(m)
    return in_maps


_BUILT = {}


def kernel(coords, w0, b0, w1, b1, w2, b2, w3, b3, w4, b4, w_last, b_last):
    key = ROWS
    if key not in _BUILT:
        _BUILT[key] = _build(ROWS)
    nc = _BUILT[key]
    in_maps = _host_prep(coords, w0, w1, w2, w3, w4, w_last, ROWS)
    res = run_bass_kernel_spmd(nc, in_maps, list(range(NCORES)), trace=TRACE)
    LAST["res"] = res
    out = np.empty((N, 3), np.float32)
    for c in range(NCORES):
        out[c * ROWS:(c + 1) * ROWS, :] = res.results[c]["out"].T
    return out


# revision 32
# speedup vs baseline: 1.1999x; 1.1999x over previous
"""Trainium2 Bass kernel for the LoE tiled-MLP (NeRF-style coordinate net).

Sharding: data-parallel over the pixel axis. N=262144 rows are split
contiguously across 8 cores (32768 rows each). Because the per-layer
expert tiles are contiguous row blocks, each core only ever needs a
contiguous slice of every weight tensor -> zero cross-core traffic.

On-device layout: activations are feature-major [d, n]; every layer is
psum[o, n] += w[d_blk, o_blk].T @ x[d_blk, n] with w as the stationary
operand.  All layer matmuls are bf16 (tolerance 2e-2 >> bf16's ~6e-3).

The kernel keeps the PE busy ~100% of the time: TRN2's HAM clock gate
halves the PE clock (2.4 -> 1.2 GHz) whenever the PE has an idle 3.4us
window, so every PE bubble costs double.  Structure:
  * chunks run in groups of 8, layer-batched: the in-order PE queue
    holds ~6us of independent matmuls behind any wait on the LeakyReLU
    chain, and the 4-slot PSUM ring gives ~3.5us of slack before a
    slot-reuse wait can bite (worst lrelu latency ~2.8us)
  * pos-enc for group g+1 (angle matmul -> magic-round/frac on DVE ->
    sin on ACT) is pipelined into group g two chunks at a time, packed
    at partition offsets 0/64 so rnd/frac/sin are one op per 2 chunks,
    with the sin ops spread across the group so they never
    head-of-line-block the ACT queue
  * coords are pre-split on the host into three bf16 components
    (c = hi + mid + lo, exact to fp32 precision), so the angle matmul
    is one K=7 bf16 matmul instead of a 4x-slower fp32 one
  * the final [3,512] outputs land 3-to-a-psum-bank at partition
    offsets 0/32/64 (matmul tile_position), and the `last` batch is
    emitted one group late so its lrelu inputs are long since ready
  * LeakyReLU: 32 of 40 tiles/group as single ACT Prelu ops (sin and
    parametric_relu share the trig_and_small table set), 8 on DVE
    (2-op), spread 1-2 per layer so neither engine's in-order queue
    falls behind the PE's layer cadence
"""

import os
import sys

import numpy as np

sys.path.insert(0, "/opt/trn_rl_repo")

import ml_dtypes

import concourse.bass as bass
import concourse.bacc as bacc
import concourse.mybir as mybir
import concourse.tile as tile
from concourse.alu_op_type import AluOpType
from concourse.bass_utils import run_bass_kernel_spmd

F32 = mybir.dt.float32
BF16 = mybir.dt.bfloat16
ACT_SIN = mybir.ActivationFunctionType.Sin
ACT_PRELU = mybir.ActivationFunctionType.Prelu

N = 262144
NCORES = 8
ROWS = N // NCORES          # 32768 rows per core
CH = 512                    # pixels per chunk (psum free-dim, fp32 max)
G = 8                       # chunks per group
K = 13                      # frequencies
H = 256
PE_SC = 2 * 2 * K + 2       # 52 sin/cos + 2 linearized coord rows
CROWS = 7                   # coord rows: hi_x hi_y mid_x mid_y lo_x lo_y one
COORD_S = float(2.0 ** -11)  # tiny freq: sin(2*pi*s*c) ~ 2*pi*s*c
MAGIC = float(1.5 * 2 ** 23)
TWO_PI = float(2.0 * np.pi)

# which chunks of each layer's 8-chunk batch run LeakyReLU on DVE (2-op)
# instead of ACT (1-op Prelu).  8 of 40 tiles go to DVE; spread so the
# DVE queue never runs so deep that a PSUM-slot-reuse wait (ring of 4)
# stalls the PE.  Empirically tuned -- larger DVE shares and other
# spreads measured slower (356-377us vs 354us).
V_CHUNK = {0: (0, 4), 1: (2, 6), 2: (3,), 3: (3,), 4: (0, 2)}

# local (per-core) expert-tile row extents for layers 1..4
TILE_ROWS = {1: 65536, 2: 16384, 3: 4096, 4: 1024}

TRACE = False
LAST = {}


def _build(rows):
    """Build the SPMD single-core Bass program for `rows` pixels."""
    nchunks = rows // CH
    ngroups = nchunks // G
    stage = G * CH // 2                          # coord cols per group
    ntile = {l: max(rows // TILE_ROWS[l], 1) for l in (1, 2, 3, 4)}
    tidx = {l: [min(j * CH // TILE_ROWS[l], ntile[l] - 1) for j in range(nchunks)]
            for l in (1, 2, 3, 4)}

    nc = bacc.Bacc()
    d_coords = nc.dram_tensor("coordsB", [2 * CROWS, rows // 2], BF16,
                              kind="ExternalInput")
    d_smat = nc.dram_tensor("smat", [2 * CROWS, 64 + PE_SC], BF16,
                            kind="ExternalInput")
    d_w0s = nc.dram_tensor("w0s", [PE_SC, H], BF16, kind="ExternalInput")
    d_wmid = {l: nc.dram_tensor(f"w{l}", [ntile[l], H, H], BF16, kind="ExternalInput")
              for l in (1, 2, 3, 4)}
    d_wl = nc.dram_tensor("wlT", [H, 3], BF16, kind="ExternalInput")
    d_out = nc.dram_tensor("out", [3, rows], F32, kind="ExternalOutput")

    with tile.TileContext(nc) as tc:
        with (
            tc.tile_pool(name="wp", bufs=1) as wp,
            tc.tile_pool(name="io", bufs=3) as iop,
            tc.tile_pool(name="sp", bufs=10) as scp,    # sc pair tiles
            tc.tile_pool(name="fp", bufs=4) as frp,    # frac pair tiles
            tc.tile_pool(name="xp", bufs=8) as xp,     # activations
            tc.tile_pool(name="mp", bufs=5) as mp,     # rnd/rt scratch + oc out
            tc.tile_pool(name="ps", bufs=4, space="PSUM") as pp,
        ):
            # ---- resident weights (DMA once, first-use order) ----
            smat_sb = wp.tile([2 * CROWS, 64 + PE_SC], BF16, tag="smat")
            nc.sync.dma_start(out=smat_sb[:], in_=d_smat[:])
            cr0 = iop.tile([2 * CROWS, stage], BF16, tag="cr")
            nc.sync.dma_start(out=cr0[:], in_=d_coords[:, 0:stage])
            # w0s duplicated at partition offsets 0 and 64 so both chunks of
            # a packed sc pair tile can feed layer-0 matmuls.
            w0s_sb = wp.tile([64 + PE_SC, H], BF16, tag="w0s")
            nc.sync.dma_start(out=w0s_sb[0:PE_SC, :], in_=d_w0s[:])
            nc.sync.dma_start(out=w0s_sb[64:64 + PE_SC, :], in_=d_w0s[:])
            wl_sb = []
            for kb in range(2):
                t = wp.tile([128, 3], BF16, tag=f"wl{kb}")
                nc.sync.dma_start(out=t[:], in_=d_wl[kb * 128:(kb + 1) * 128, :])
                wl_sb.append(t)
            cr = {}          # group -> coords tile
            sc = {}          # (group, half) -> packed sc tile [64+PE_SC, CH]
            xs = {}          # chunk -> current activation tile

            def load_coords(g):
                t = iop.tile([2 * CROWS, stage], BF16, tag="cr")
                nc.sync.dma_start(out=t[:], in_=d_coords[:, g * stage:(g + 1) * stage])
                cr[g] = t

            def posenc_mm_rf(g, half):
                """Angle matmuls + round/frac for chunks (2*half, 2*half+1) of
                group g, packed at partition offsets 0/64 of one psum bank so
                rnd and frac are a single DVE op per pair."""
                tps = pp.tile([64 + PE_SC, CH], F32, tag="lps")
                rc = cr[g][:, half * CH:(half + 1) * CH]
                nc.tensor.matmul(tps[:], smat_sb[:], rc, start=True, stop=True)
                rnd = mp.tile([64 + PE_SC, CH], F32, tag="rnd")
                nc.vector.tensor_scalar(rnd[:], tps[:], MAGIC, MAGIC,
                                        AluOpType.add, AluOpType.subtract)
                fr = frp.tile([64 + PE_SC, CH], F32, tag="fr")
                nc.vector.tensor_tensor(fr[:], tps[:], rnd[:], AluOpType.subtract)
                return fr

            def posenc_sin(g, half, fr):
                s = scp.tile([64 + PE_SC, CH], BF16, tag="sc")
                nc.scalar.activation(s[:], fr[:], ACT_SIN, scale=TWO_PI)
                sc[(g, half)] = s

            def lrelu(eng, xt, ps):
                if eng == "v":
                    rt = mp.tile([128, 2 * CH], F32, tag="rt")
                    nc.vector.tensor_scalar(rt[:], ps[:], 0.0, 0.8,
                                            AluOpType.max, AluOpType.mult)
                    nc.vector.scalar_tensor_tensor(xt[:], ps[:], 0.2, rt[:],
                                                   AluOpType.mult, AluOpType.add)
                else:
                    nc.scalar.activation(xt[:], ps[:], ACT_PRELU, alpha=0.2)

            def layer0(js):
                for i, j in enumerate(js):
                    s = sc[(j // G, i // 2)]
                    base = 64 * (i % 2)
                    ps = pp.tile([128, 2 * CH], F32, tag="lps")
                    for ob in range(2):
                        nc.tensor.matmul(ps[:, ob * CH:(ob + 1) * CH],
                                         w0s_sb[base:base + PE_SC,
                                                ob * 128:(ob + 1) * 128],
                                         s[base:base + PE_SC, :],
                                         start=True, stop=True)
                    x = xp.tile([128, 2 * CH], BF16, tag="x0")
                    lrelu("v" if i in V_CHUNK[0] else "p", x, ps)
                    xs[j] = x

            def layer(l, js):
                for i, j in enumerate(js):
                    wt = wmid_sb[l][tidx[l][j]]
                    ps = pp.tile([128, 2 * CH], F32, tag="lps")
                    for ob in range(2):
                        osl = slice(ob * CH, (ob + 1) * CH)
                        wsl = slice(ob * 128, (ob + 1) * 128)
                        for kb in range(2):
                            nc.tensor.matmul(
                                ps[:, osl], wt[kb][:, wsl],
                                xs[j][:, kb * CH:(kb + 1) * CH],
                                start=(kb == 0), stop=(kb == 1))
                    x = xp.tile([128, 2 * CH], BF16, tag=f"x{l}")
                    lrelu("v" if i in V_CHUNK[l] else "p", x, ps)
                    xs[j] = x

            def last(js, base):
                """Final [3,512] for chunks js[base:base+3], packed
                3-to-a-psum-bank at partition offsets 0/32/64 (matmul
                tile_position) -> one copy, 3 output DMAs.  Called at three
                separate emission points so neither the PE matmuls nor the
                DVE copies bunch up."""
                sub = js[base:base + 3]
                po = pp.tile([32 * (len(sub) - 1) + 3, CH], F32, tag="lps")
                for i, j in enumerate(sub):
                    for kb in range(2):
                        nc.tensor.matmul(po[32 * i:32 * i + 3, :], wl_sb[kb][:],
                                         xs[j][:, kb * CH:(kb + 1) * CH],
                                         start=(kb == 0), stop=(kb == 1))
                oc = mp.tile([32 * (len(sub) - 1) + 3, CH], F32, tag="oc")
                nc.vector.tensor_copy(oc[:], po[:])
                for i, j in enumerate(sub):
                    nc.sync.dma_start(out=d_out[:, j * CH:(j + 1) * CH],
                                      in_=oc[32 * i:32 * i + 3, :])
                    del xs[j]

            # ---- prologue: group 0's pos-enc, then bulk weights ----
            # Coords + pos-enc are emitted BEFORE the 86 expert-weight DMAs,
            # and the weight DMAs are issued from the (otherwise idle) GpSimd
            # queue: ~650ns of descriptor-issue each would otherwise hold the
            # first matmul back by ~60us of Sync-queue serialization.  The
            # group-0 coords DMA was already issued right behind smat above.
            cr[0] = cr0
            for h in range(4):
                fr = posenc_mm_rf(0, h)
                posenc_sin(0, h, fr)

            wmid_sb = {l: [[None, None] for _ in range(ntile[l])] for l in (1, 2, 3, 4)}
            order = []
            for l in (1, 2, 3, 4):
                for t in range(ntile[l]):
                    first = min(j for j in range(nchunks) if tidx[l][j] == t)
                    order.append((first, l, t))
            order.sort()
            for _, l, t in order:
                for kb in range(2):
                    w = wp.tile([128, H], BF16, tag=f"w{l}_{t}_{kb}")
                    nc.gpsimd.dma_start(
                        out=w[:], in_=d_wmid[l][t, kb * 128:(kb + 1) * 128, :])
                    wmid_sb[l][t][kb] = w

            frs = {}
            for g in range(ngroups):
                js = list(range(g * G, (g + 1) * G))
                pj = list(range((g - 1) * G, g * G))     # previous group
                nxt = g + 1 < ngroups
                if nxt:
                    load_coords(g + 1)
                    frs[0] = posenc_mm_rf(g + 1, 0)
                layer0(js)
                if g > 0:
                    last(pj, 0)
                if nxt:
                    posenc_sin(g + 1, 0, frs[0])
                    frs[1] = posenc_mm_rf(g + 1, 1)
                layer(1, js)
                if g > 0:
                    last(pj, 3)
                if nxt:
                    posenc_sin(g + 1, 1, frs[1])
                    frs[2] = posenc_mm_rf(g + 1, 2)
                layer(2, js)
                if g > 0:
                    last(pj, 6)
                if nxt:
                    posenc_sin(g + 1, 2, frs[2])
                    frs[3] = posenc_mm_rf(g + 1, 3)
                layer(3, js)
                if nxt:
                    posenc_sin(g + 1, 3, frs[3])
                    del cr[g]
                layer(4, js)
                del sc[(g, 0)], sc[(g, 1)], sc[(g, 2)], sc[(g, 3)]
            fj = list(range((ngroups - 1) * G, ngroups * G))
            for b in (0, 3, 6):
                last(fj, b)
    nc.finalize()
    return nc


def _host_prep(coords, w0, w1, w2, w3, w4, w_last, rows):
    """Split full inputs into per-core in_maps."""
    coords = np.asarray(coords, np.float32)
    bf = ml_dtypes.bfloat16
    smat = np.zeros((CROWS, PE_SC), np.float32)
    for p in range(PE_SC - 2):
        k, f, s = p >> 2, (p >> 1) & 1, p & 1
        for piece in range(3):
            smat[2 * piece + f, p] = float(2.0 ** (k - 1))
        smat[6, p] = 0.25 if s else 0.0
    for piece in range(3):
        smat[2 * piece + 0, PE_SC - 2] = COORD_S
        smat[2 * piece + 1, PE_SC - 1] = COORD_S
    w0 = np.asarray(w0, np.float32)[0]              # [54, 256]
    w0s = np.empty((PE_SC, H), np.float32)
    w0s[:PE_SC - 2] = w0[2:]
    w0s[PE_SC - 2:] = w0[0:2] / np.float32(2.0 * np.pi * COORD_S)
    wlT = np.ascontiguousarray(np.asarray(w_last, np.float32).T)  # [256, 3]
    wmid_full = {1: np.asarray(w1, np.float32), 2: np.asarray(w2, np.float32),
                 3: np.asarray(w3, np.float32), 4: np.asarray(w4, np.float32)}
    ntile = {l: max(rows // TILE_ROWS[l], 1) for l in (1, 2, 3, 4)}

    # triple bf16 split of coords: c = hi + mid + lo, exact to ~fp32
    cT = coords.T                                   # [2, N]
    hi = cT.astype(bf)
    r1 = cT - hi.astype(np.float32)
    mid = r1.astype(bf)
    lo = (r1 - mid.astype(np.float32)).astype(bf)

    # block-diagonal smat: one matmul computes both partition-packed
    # halves of a chunk pair (even chunk rows 0-6 -> out 0-53, odd chunk
    # rows 7-13 -> out 64-117)
    smat14 = np.zeros((2 * CROWS, 64 + PE_SC), np.float32)
    smat14[0:CROWS, 0:PE_SC] = smat
    smat14[CROWS:, 64:64 + PE_SC] = smat
    in_maps = []
    for c in range(NCORES):
        sl = slice(c * rows, (c + 1) * rows)
        cb = np.empty((CROWS, rows), bf)
        cb[0:2] = hi[:, sl]
        cb[2:4] = mid[:, sl]
        cb[4:6] = lo[:, sl]
        cb[6] = np.float32(1.0)
        cv = cb.reshape(CROWS, rows // (2 * CH), 2, CH)
        cb14 = np.concatenate((cv[:, :, 0, :], cv[:, :, 1, :]),
                              axis=0).reshape(2 * CROWS, rows // 2)
        m = {"coordsB": np.ascontiguousarray(cb14), "smat": smat14.astype(bf),
             "w0s": w0s.astype(bf), "wlT": wlT.astype(bf)}
        for l in (1, 2, 3, 4):
            w = wmid_full[l]
            t0 = (c * rows) // (N // w.shape[0])
            m[f"w{l}"] = np.ascontiguousarray(w[t0:t0 + ntile[l]]).astype(bf)
        in_maps.append(m)
    return in_maps


_BUILT = {}


def kernel(coords, w0, b0, w1, b1, w2, b2, w3, b3, w4, b4, w_last, b_last):
    key = ROWS
    if key not in _BUILT:
        _BUILT[key] = _build(ROWS)
    nc = _BUILT[key]
    in_maps = _host_prep(coords, w0, w1, w2, w3, w4, w_last, ROWS)
    res = run_bass_kernel_spmd(nc, in_maps, list(range(NCORES)), trace=TRACE)
    LAST["res"] = res
    out = np.empty((N, 3), np.float32)
    for c in range(NCORES):
        out[c * ROWS:(c + 1) * ROWS, :] = res.results[c]["out"].T
    return out


# revision 34
# speedup vs baseline: 1.2001x; 1.0001x over previous
"""Trainium2 Bass kernel for the LoE tiled-MLP (NeRF-style coordinate net).

Sharding: data-parallel over the pixel axis. N=262144 rows are split
contiguously across 8 cores (32768 rows each). Because the per-layer
expert tiles are contiguous row blocks, each core only ever needs a
contiguous slice of every weight tensor -> zero cross-core traffic.

On-device layout: activations are feature-major [d, n]; every layer is
psum[o, n] += w[d_blk, o_blk].T @ x[d_blk, n] with w as the stationary
operand.  All layer matmuls are bf16 (tolerance 2e-2 >> bf16's ~6e-3).

The kernel keeps the PE busy ~100% of the time: TRN2's HAM clock gate
halves the PE clock (2.4 -> 1.2 GHz) whenever the PE has an idle 3.4us
window, so every PE bubble costs double.  Structure:
  * chunks run in groups of 8, layer-batched: the in-order PE queue
    holds ~6us of independent matmuls behind any wait on the LeakyReLU
    chain, and the 4-slot PSUM ring gives ~3.5us of slack before a
    slot-reuse wait can bite (worst lrelu latency ~2.8us)
  * pos-enc for group g+1 (angle matmul -> magic-round/frac on DVE ->
    sin on ACT) is pipelined into group g two chunks at a time, packed
    at partition offsets 0/64 so rnd/frac/sin are one op per 2 chunks,
    with the sin ops spread across the group so they never
    head-of-line-block the ACT queue
  * coords are pre-split on the host into three bf16 components
    (c = hi + mid + lo, exact to fp32 precision), so the angle matmul
    is one K=7 bf16 matmul instead of a 4x-slower fp32 one
  * the final [3,512] outputs land 3-to-a-psum-bank at partition
    offsets 0/32/64 (matmul tile_position), and the `last` batch is
    emitted one group late so its lrelu inputs are long since ready
  * LeakyReLU: 32 of 40 tiles/group as single ACT Prelu ops (sin and
    parametric_relu share the trig_and_small table set), 8 on DVE
    (2-op), spread 1-2 per layer so neither engine's in-order queue
    falls behind the PE's layer cadence
"""

import os
import sys

import numpy as np

sys.path.insert(0, "/opt/trn_rl_repo")

import ml_dtypes

import concourse.bass as bass
import concourse.bacc as bacc
import concourse.mybir as mybir
import concourse.tile as tile
from concourse.alu_op_type import AluOpType
from concourse.bass_utils import run_bass_kernel_spmd

F32 = mybir.dt.float32
BF16 = mybir.dt.bfloat16
ACT_SIN = mybir.ActivationFunctionType.Sin
ACT_PRELU = mybir.ActivationFunctionType.Prelu

N = 262144
NCORES = 8
ROWS = N // NCORES          # 32768 rows per core
CH = 512                    # pixels per chunk (psum free-dim, fp32 max)
G = 8                       # chunks per group
K = 13                      # frequencies
H = 256
PE_SC = 2 * 2 * K + 2       # 52 sin/cos + 2 linearized coord rows
CROWS = 7                   # coord rows: hi_x hi_y mid_x mid_y lo_x lo_y one
COORD_S = float(2.0 ** -11)  # tiny freq: sin(2*pi*s*c) ~ 2*pi*s*c
MAGIC = float(1.5 * 2 ** 23)
TWO_PI = float(2.0 * np.pi)

# which chunks of each layer's 8-chunk batch run LeakyReLU on DVE (2-op)
# instead of ACT (1-op Prelu).  8 of 40 tiles go to DVE; spread so the
# DVE queue never runs so deep that a PSUM-slot-reuse wait (ring of 4)
# stalls the PE.  Empirically tuned -- larger DVE shares and other
# spreads measured slower (356-377us vs 354us).
V_CHUNK = {0: (0, 4), 1: (2, 6), 2: (3,), 3: (3,), 4: (0, 2)}

# local (per-core) expert-tile row extents for layers 1..4
TILE_ROWS = {1: 65536, 2: 16384, 3: 4096, 4: 1024}

TRACE = False
LAST = {}


def _build(rows):
    """Build the SPMD single-core Bass program for `rows` pixels."""
    nchunks = rows // CH
    ngroups = nchunks // G
    stage = G * CH // 2                          # coord cols per group
    ntile = {l: max(rows // TILE_ROWS[l], 1) for l in (1, 2, 3, 4)}
    tidx = {l: [min(j * CH // TILE_ROWS[l], ntile[l] - 1) for j in range(nchunks)]
            for l in (1, 2, 3, 4)}

    nc = bacc.Bacc()
    d_coords = nc.dram_tensor("coordsB", [2 * CROWS, rows // 2], BF16,
                              kind="ExternalInput")
    d_smat = nc.dram_tensor("smat", [2 * CROWS, 64 + PE_SC], BF16,
                            kind="ExternalInput")
    d_w0s = nc.dram_tensor("w0s", [PE_SC, H], BF16, kind="ExternalInput")
    d_wmid = {l: nc.dram_tensor(f"w{l}", [ntile[l], H, H], BF16, kind="ExternalInput")
              for l in (1, 2, 3, 4)}
    d_wl = nc.dram_tensor("wlT", [H, 3], BF16, kind="ExternalInput")
    d_out = nc.dram_tensor("out", [3, rows], F32, kind="ExternalOutput")

    with tile.TileContext(nc) as tc:
        with (
            tc.tile_pool(name="wp", bufs=1) as wp,
            tc.tile_pool(name="io", bufs=3) as iop,
            tc.tile_pool(name="sp", bufs=10) as scp,    # sc pair tiles
            tc.tile_pool(name="fp", bufs=4) as frp,    # frac pair tiles
            tc.tile_pool(name="xp", bufs=8) as xp,     # activations
            tc.tile_pool(name="mp", bufs=5) as mp,     # rnd/rt scratch + oc out
            tc.tile_pool(name="ps", bufs=4, space="PSUM") as pp,
        ):
            # ---- resident weights (DMA once, first-use order) ----
            smat_sb = wp.tile([2 * CROWS, 64 + PE_SC], BF16, tag="smat")
            nc.sync.dma_start(out=smat_sb[:], in_=d_smat[:])
            cr0 = iop.tile([2 * CROWS, stage], BF16, tag="cr")
            nc.sync.dma_start(out=cr0[:], in_=d_coords[:, 0:stage])
            # w0s duplicated at partition offsets 0 and 64 so both chunks of
            # a packed sc pair tile can feed layer-0 matmuls.
            w0s_sb = wp.tile([64 + PE_SC, H], BF16, tag="w0s")
            nc.sync.dma_start(out=w0s_sb[0:PE_SC, :], in_=d_w0s[:])
            nc.sync.dma_start(out=w0s_sb[64:64 + PE_SC, :], in_=d_w0s[:])
            wl_sb = []
            for kb in range(2):
                t = wp.tile([128, 3], BF16, tag=f"wl{kb}")
                nc.sync.dma_start(out=t[:], in_=d_wl[kb * 128:(kb + 1) * 128, :])
                wl_sb.append(t)
            cr = {}          # group -> coords tile
            sc = {}          # (group, half) -> packed sc tile [64+PE_SC, CH]
            xs = {}          # chunk -> current activation tile

            def load_coords(g):
                t = iop.tile([2 * CROWS, stage], BF16, tag="cr")
                nc.sync.dma_start(out=t[:], in_=d_coords[:, g * stage:(g + 1) * stage])
                cr[g] = t

            def posenc_mm_rf(g, half):
                """Angle matmuls + round/frac for chunks (2*half, 2*half+1) of
                group g, packed at partition offsets 0/64 of one psum bank so
                rnd and frac are a single DVE op per pair."""
                tps = pp.tile([64 + PE_SC, CH], F32, tag="lps")
                rc = cr[g][:, half * CH:(half + 1) * CH]
                nc.tensor.matmul(tps[:], smat_sb[:], rc, start=True, stop=True)
                rnd = mp.tile([64 + PE_SC, CH], F32, tag="rnd")
                nc.vector.tensor_scalar(rnd[:], tps[:], MAGIC, MAGIC,
                                        AluOpType.add, AluOpType.subtract)
                fr = frp.tile([64 + PE_SC, CH], F32, tag="fr")
                nc.vector.tensor_tensor(fr[:], tps[:], rnd[:], AluOpType.subtract)
                return fr

            def posenc_sin(g, half, fr):
                s = scp.tile([64 + PE_SC, CH], BF16, tag="sc")
                nc.scalar.activation(s[:], fr[:], ACT_SIN, scale=TWO_PI)
                sc[(g, half)] = s

            def lrelu(eng, xt, ps):
                if eng == "v":
                    rt = mp.tile([128, 2 * CH], F32, tag="rt")
                    nc.vector.tensor_scalar(rt[:], ps[:], 0.0, 0.8,
                                            AluOpType.max, AluOpType.mult)
                    nc.vector.scalar_tensor_tensor(xt[:], ps[:], 0.2, rt[:],
                                                   AluOpType.mult, AluOpType.add)
                else:
                    nc.scalar.activation(xt[:], ps[:], ACT_PRELU, alpha=0.2)

            def layer0(js):
                for i, j in enumerate(js):
                    s = sc[(j // G, i // 2)]
                    base = 64 * (i % 2)
                    ps = pp.tile([128, 2 * CH], F32, tag="lps")
                    for ob in range(2):
                        nc.tensor.matmul(ps[:, ob * CH:(ob + 1) * CH],
                                         w0s_sb[base:base + PE_SC,
                                                ob * 128:(ob + 1) * 128],
                                         s[base:base + PE_SC, :],
                                         start=True, stop=True)
                    x = xp.tile([128, 2 * CH], BF16, tag="x0")
                    lrelu("v" if i in V_CHUNK[0] else "p", x, ps)
                    xs[j] = x

            def layer(l, js):
                for i, j in enumerate(js):
                    wt = wmid_sb[l][tidx[l][j]]
                    ps = pp.tile([128, 2 * CH], F32, tag="lps")
                    for ob in range(2):
                        osl = slice(ob * CH, (ob + 1) * CH)
                        wsl = slice(ob * 128, (ob + 1) * 128)
                        for kb in range(2):
                            nc.tensor.matmul(
                                ps[:, osl], wt[kb][:, wsl],
                                xs[j][:, kb * CH:(kb + 1) * CH],
                                start=(kb == 0), stop=(kb == 1))
                    x = xp.tile([128, 2 * CH], BF16, tag=f"x{l}")
                    lrelu("v" if i in V_CHUNK[l] else "p", x, ps)
                    xs[j] = x

            def last(js, base):
                """Final [3,512] for chunks js[base:base+3], packed
                3-to-a-psum-bank at partition offsets 0/32/64 (matmul
                tile_position) -> one copy, 3 output DMAs.  Called at three
                separate emission points so neither the PE matmuls nor the
                DVE copies bunch up."""
                sub = js[base:base + 3]
                po = pp.tile([32 * (len(sub) - 1) + 3, CH], F32, tag="lps")
                for i, j in enumerate(sub):
                    for kb in range(2):
                        nc.tensor.matmul(po[32 * i:32 * i + 3, :], wl_sb[kb][:],
                                         xs[j][:, kb * CH:(kb + 1) * CH],
                                         start=(kb == 0), stop=(kb == 1))
                oc = mp.tile([32 * (len(sub) - 1) + 3, CH], F32, tag="oc")
                nc.vector.tensor_copy(oc[:], po[:])
                for i, j in enumerate(sub):
                    nc.sync.dma_start(out=d_out[:, j * CH:(j + 1) * CH],
                                      in_=oc[32 * i:32 * i + 3, :])
                    del xs[j]

            # ---- prologue: group 0's pos-enc, then bulk weights ----
            # Coords + pos-enc are emitted BEFORE the 86 expert-weight DMAs,
            # and the weight DMAs are issued from the (otherwise idle) GpSimd
            # queue: ~650ns of descriptor-issue each would otherwise hold the
            # first matmul back by ~60us of Sync-queue serialization.  The
            # group-0 coords DMA was already issued right behind smat above.
            cr[0] = cr0
            for h in range(4):
                fr = posenc_mm_rf(0, h)
                posenc_sin(0, h, fr)

            wmid_sb = {l: [[None, None] for _ in range(ntile[l])] for l in (1, 2, 3, 4)}
            order = []
            for l in (1, 2, 3, 4):
                for t in range(ntile[l]):
                    first = min(j for j in range(nchunks) if tidx[l][j] == t)
                    order.append((first, l, t))
            order.sort()
            for _, l, t in order:
                for kb in range(2):
                    w = wp.tile([128, H], BF16, tag=f"w{l}_{t}_{kb}")
                    nc.gpsimd.dma_start(
                        out=w[:], in_=d_wmid[l][t, kb * 128:(kb + 1) * 128, :])
                    wmid_sb[l][t][kb] = w

            frs = {}
            for g in range(ngroups):
                js = list(range(g * G, (g + 1) * G))
                pj = list(range((g - 1) * G, g * G))     # previous group
                nxt = g + 1 < ngroups
                if nxt:
                    load_coords(g + 1)
                    frs[0] = posenc_mm_rf(g + 1, 0)
                layer0(js)
                if g > 0:
                    last(pj, 0)
                if nxt:
                    posenc_sin(g + 1, 0, frs[0])
                    frs[1] = posenc_mm_rf(g + 1, 1)
                layer(1, js)
                if g > 0:
                    last(pj, 3)
                if nxt:
                    posenc_sin(g + 1, 1, frs[1])
                    frs[2] = posenc_mm_rf(g + 1, 2)
                layer(2, js)
                if g > 0:
                    last(pj, 6)
                if nxt:
                    posenc_sin(g + 1, 2, frs[2])
                    frs[3] = posenc_mm_rf(g + 1, 3)
                layer(3, js)
                if nxt:
                    posenc_sin(g + 1, 3, frs[3])
                    del cr[g]
                layer(4, js)
                del sc[(g, 0)], sc[(g, 1)], sc[(g, 2)], sc[(g, 3)]
            fj = list(range((ngroups - 1) * G, ngroups * G))
            for b in (0, 3, 6):
                last(fj, b)
    nc.finalize()
    return nc


def _host_prep(coords, w0, w1, w2, w3, w4, w_last, rows):
    """Split full inputs into per-core in_maps."""
    coords = np.asarray(coords, np.float32)
    bf = ml_dtypes.bfloat16
    smat = np.zeros((CROWS, PE_SC), np.float32)
    for p in range(PE_SC - 2):
        k, f, s = p >> 2, (p >> 1) & 1, p & 1
        for piece in range(3):
            smat[2 * piece + f, p] = float(2.0 ** (k - 1))
        smat[6, p] = 0.25 if s else 0.0
    for piece in range(3):
        smat[2 * piece + 0, PE_SC - 2] = COORD_S
        smat[2 * piece + 1, PE_SC - 1] = COORD_S
    w0 = np.asarray(w0, np.float32)[0]              # [54, 256]
    w0s = np.empty((PE_SC, H), np.float32)
    w0s[:PE_SC - 2] = w0[2:]
    w0s[PE_SC - 2:] = w0[0:2] / np.float32(2.0 * np.pi * COORD_S)
    wlT = np.ascontiguousarray(np.asarray(w_last, np.float32).T)  # [256, 3]
    wmid_full = {1: np.asarray(w1, np.float32), 2: np.asarray(w2, np.float32),
                 3: np.asarray(w3, np.float32), 4: np.asarray(w4, np.float32)}
    ntile = {l: max(rows // TILE_ROWS[l], 1) for l in (1, 2, 3, 4)}

    # triple bf16 split of coords: c = hi + mid + lo, exact to ~fp32
    cT = coords.T                                   # [2, N]
    hi = cT.astype(bf)
    r1 = cT - hi.astype(np.float32)
    mid = r1.astype(bf)
    lo = (r1 - mid.astype(np.float32)).astype(bf)

    # block-diagonal smat: one matmul computes both partition-packed
    # halves of a chunk pair (even chunk rows 0-6 -> out 0-53, odd chunk
    # rows 7-13 -> out 64-117)
    smat14 = np.zeros((2 * CROWS, 64 + PE_SC), np.float32)
    smat14[0:CROWS, 0:PE_SC] = smat
    smat14[CROWS:, 64:64 + PE_SC] = smat
    in_maps = []
    for c in range(NCORES):
        sl = slice(c * rows, (c + 1) * rows)
        cb = np.empty((CROWS, rows), bf)
        cb[0:2] = hi[:, sl]
        cb[2:4] = mid[:, sl]
        cb[4:6] = lo[:, sl]
        cb[6] = np.float32(1.0)
        cv = cb.reshape(CROWS, rows // (2 * CH), 2, CH)
        cb14 = np.concatenate((cv[:, :, 0, :], cv[:, :, 1, :]),
                              axis=0).reshape(2 * CROWS, rows // 2)
        m = {"coordsB": np.ascontiguousarray(cb14), "smat": smat14.astype(bf),
             "w0s": w0s.astype(bf), "wlT": wlT.astype(bf)}
        for l in (1, 2, 3, 4):
            w = wmid_full[l]
            t0 = (c * rows) // (N // w.shape[0])
            m[f"w{l}"] = np.ascontiguousarray(w[t0:t0 + ntile[l]]).astype(bf)
        in_maps.append(m)
    return in_maps


_BUILT = {}


def kernel(coords, w0, b0, w1, b1, w2, b2, w3, b3, w4, b4, w_last, b_last):
    key = ROWS
    if key not in _BUILT:
        _BUILT[key] = _build(ROWS)
    nc = _BUILT[key]
    in_maps = _host_prep(coords, w0, w1, w2, w3, w4, w_last, ROWS)
    res = run_bass_kernel_spmd(nc, in_maps, list(range(NCORES)), trace=TRACE)
    LAST["res"] = res
    out = np.empty((N, 3), np.float32)
    for c in range(NCORES):
        out[c * ROWS:(c + 1) * ROWS, :] = res.results[c]["out"].T
    return out
